# revision 12
# baseline (speedup 1.0000x reference)
"""Trainium2 Bass kernel for nn_DivMergedLayer1 (dense_mlp, memory-bound).

The baked FFN weights are ultra-sparse: the whole module reduces to
``out = x`` everywhere except four scalars per batch row::

    op   = x[b, 0, 67]                      (opcode channel, >= 0)
    sg   = sum_i f32(f32(60*op) * f32(2^i * x[b, i, 0])) / 60
    s2   = sum_i max((x[b,i,1] > 0.5) * (2^i * x[b,i,1]), exp(-60))
    out[b, 0, k] = x[b,0,k] + f32(60*op * x[b,0,k]) * (-1/60)   k in {2,3,4,5}
    out[b, 0, 2] += sg
    out[b, 0, 5] += op / s2

So the kernel is a memory-bound copy with a tiny per-row fixup.  The
output tolerance (2e-2 scale-relative on an output whose absmax is ~4e9)
lets the bulk copy ride in bfloat16: the host downcasts x once, the
device streams the bf16 image back out (halving HBM traffic vs f32),
and the four genuinely-computed scalars per row travel in a small f32
side-channel (exact slices of x) so the fixup math is bit-comparable to
a full-f32 kernel.  The host merges the f32 fixup column block into the
upcast bf16 copy.  Pure data parallel over the batch axis; 1024 rows
per core; ~17 MB of HBM traffic per core vs 33.5 MB for the f32 kernel.
"""

import math

import numpy as np

N_CORES = 8
B, N, D = 8192, 32, 128
F = N * D                  # 4096 flattened features per row
R = B // N_CORES           # 1024 rows per core
P = 128                    # SBUF partitions
RPP = R // P               # 8 rows per partition
C = 2 * N + 1 + 4          # side-channel floats per row: a[32] d[32] op slots[4]

OP_COL = 67                # flat index of opcode channel (pos 0, feat 64+3)

_INV_S = float(np.float32(1.0 / 60.0))
_NEG_INV_S = float(np.float32(-1.0 / 60.0))
_EXP_NEG60 = float(np.float32(math.exp(-60.0)))

VARIANT = "d2d"    # DRAM->DRAM bulk copy, chunks alternating sync/scalar rings
TILES = 8          # copy DMA chunks (must divide RPP*F partition free dim)

_COMPILED = None


def _build(variant=VARIANT, tiles=TILES):
    import concourse.bacc as bacc
    import concourse.mybir as mybir
    from concourse.tile import TileContext

    f32 = mybir.dt.float32
    bf16 = mybir.dt.bfloat16
    mult = mybir.AluOpType.mult
    add = mybir.AluOpType.add
    is_gt = mybir.AluOpType.is_gt
    amax = mybir.AluOpType.max

    nc = bacc.Bacc(
        "TRN2", target_bir_lowering=False, debug=False, num_devices=N_CORES
    )
    xb_h = nc.dram_tensor("xb", [R, F], bf16, kind="ExternalInput")
    sc_h = nc.dram_tensor("sc", [P, RPP * C], f32, kind="ExternalInput")
    pw_h = nc.dram_tensor("pw", [P, N], f32, kind="ExternalInput")
    ob_h = nc.dram_tensor("ob", [R, F], bf16, kind="ExternalOutput")
    fx_h = nc.dram_tensor("fx", [P, RPP * 4], f32, kind="ExternalOutput")

    # partition p holds rows p*RPP .. p*RPP+RPP-1 (contiguous per partition)
    xv = xb_h.ap().rearrange("(p j) f -> p (j f)", p=P)
    ov = ob_h.ap().rearrange("(p j) f -> p (j f)", p=P)
    assert (RPP * F) % tiles == 0
    CH = RPP * F // tiles      # chunk free-dim elements per partition

    with TileContext(nc) as tc:
        with (
            tc.tile_pool(name="const", bufs=1) as cpool,
            tc.tile_pool(name="big", bufs=2) as bpool,
            tc.tile_pool(name="small", bufs=4) as spool,
        ):
            # const + side-channel loads go first on the scalar HWDGE ring
            # (~1 us) so DVE compute starts early; gpsimd SWDGE is avoided
            # entirely -- its slow software dispatch showed up as multi-us
            # startup/tail holes in traces
            pw = cpool.tile([P, N], f32)
            nc.scalar.dma_start(out=pw[:], in_=pw_h.ap())
            sct = cpool.tile([P, RPP, C], f32)
            nc.scalar.dma_start(
                out=sct[:], in_=sc_h.ap().rearrange("p (j c) -> p j c", c=C)
            )
            res = cpool.tile([P, RPP, 4], f32)

            # ---- bulk copy: out_bf16 = x_bf16, no compute dependency ----
            if variant == "d2dG":
                # sync/scalar take 3/8 each as `tiles` chunks; gpsimd takes
                # the last 2/8 as one single SWDGE instruction
                cut = (tiles * 3) // 8
                for t in range(tiles):
                    if t < cut:
                        eng = nc.sync
                    elif t < 2 * cut:
                        eng = nc.scalar
                    else:
                        break
                    eng.dma_start(
                        out=ov[:, t * CH:(t + 1) * CH],
                        in_=xv[:, t * CH:(t + 1) * CH],
                    )
                lo = 2 * cut * CH
                nc.gpsimd.dma_start(out=ov[:, lo:], in_=xv[:, lo:])
            elif variant.startswith("d2d"):
                engs = [nc.sync, nc.scalar]
                if variant == "d2d3":
                    engs.append(nc.gpsimd)
                elif variant == "d2d1":
                    engs = [nc.sync]
                for t in range(tiles):
                    engs[t % len(engs)].dma_start(
                        out=ov[:, t * CH:(t + 1) * CH],
                        in_=xv[:, t * CH:(t + 1) * CH],
                    )
            else:
                for t in range(tiles):
                    X = bpool.tile([P, CH], bf16, tag="X")
                    nc.sync.dma_start(out=X[:], in_=xv[:, t * CH:(t + 1) * CH])
                    nc.scalar.dma_start(
                        out=ov[:, t * CH:(t + 1) * CH], in_=X[:]
                    )

            # ---- fixup: per-row scalars from the f32 side-channel ----
            V = nc.vector
            for j in range(RPP):
                Bj = sct[:, j]
                a_ap = Bj[:, 0:N]
                d_ap = Bj[:, N:2 * N]
                op_ap = Bj[:, 2 * N:2 * N + 1]
                slots = Bj[:, 2 * N + 1:2 * N + 5]

                op60 = spool.tile([P, 1], f32, tag="op60")
                g = spool.tile([P, N], f32, tag="g")
                val = spool.tile([P, N], f32, tag="val")
                msk = spool.tile([P, N], f32, tag="msk")
                extra = spool.tile([P, 4], f32, tag="extra")
                s2 = spool.tile([P, 1], f32, tag="s2")
                s2r = spool.tile([P, 1], f32, tag="s2r")
                c4 = spool.tile([P, 4], f32, tag="c4")

                V.tensor_scalar_mul(op60[:], op_ap, 60.0)
                # gather term -> extra[:,0]
                V.tensor_tensor(g[:], a_ap, pw[:], mult)
                V.tensor_scalar_mul(g[:], g[:], op60[:])
                V.tensor_scalar(
                    g[:], g[:], _INV_S, None, mult, add,
                    accum_out=extra[:, 0:1],
                )
                # softmax1-reciprocal term -> extra[:,3]
                V.tensor_tensor(val[:], d_ap, pw[:], mult)
                V.tensor_scalar(msk[:], d_ap, 0.5, None, is_gt)
                V.tensor_tensor(val[:], val[:], msk[:], mult)
                V.tensor_scalar(
                    val[:], val[:], _EXP_NEG60, None, amax, add,
                    accum_out=s2[:],
                )
                V.reciprocal(s2r[:], s2[:])
                V.tensor_tensor(extra[:, 3:4], s2r[:], op_ap, mult)
                V.memset(extra[:, 1:3], 0.0)
                # cleared slots, matching the reference's rounding order
                V.tensor_scalar_mul(c4[:], slots, op60[:])
                V.scalar_tensor_tensor(c4[:], c4[:], _NEG_INV_S, slots, mult, add)
                V.tensor_tensor(res[:, j], c4[:], extra[:], add)
            nc.sync.dma_start(
                out=fx_h.ap(), in_=res[:].rearrange("p j c -> p (j c)")
            )
    nc.compile()
    return nc


def _get_compiled():
    global _COMPILED
    if _COMPILED is None:
        _COMPILED = _build()
    return _COMPILED


def _prep_in_maps(inputs):
    import ml_dtypes

    x = np.ascontiguousarray(np.asarray(inputs["x"], dtype=np.float32))
    assert x.shape == (B, N, D), x.shape
    xr = x.reshape(B, F)
    xb = xr.astype(ml_dtypes.bfloat16)
    bpw = np.asarray(inputs["base_powers"]).astype(np.float32)
    pw = np.ascontiguousarray(np.broadcast_to(bpw[None, :], (P, N)))
    sc = np.concatenate(
        [x[:, :, 0], x[:, :, 1], xr[:, OP_COL:OP_COL + 1], xr[:, 2:6]], axis=1
    )                                            # [B, C] f32
    in_maps = []
    for i in range(N_CORES):
        s = slice(i * R, (i + 1) * R)
        in_maps.append({
            "xb": np.ascontiguousarray(xb[s]),
            "sc": np.ascontiguousarray(sc[s]).reshape(P, RPP * C),
            "pw": pw,
        })
    return in_maps


def _assemble(results):
    outs = []
    for i in range(N_CORES):
        of = np.asarray(results[i]["ob"]).astype(np.float32)   # [R, F]
        fx = np.asarray(results[i]["fx"]).reshape(R, 4)
        of[:, 2:6] = fx
        outs.append(of)
    return np.ascontiguousarray(
        np.concatenate(outs, axis=0).reshape(B, N, D).astype(np.float32)
    )


def kernel(**inputs):
    from concourse.bass_utils import run_bass_kernel_spmd

    nc = _get_compiled()
    in_maps = _prep_in_maps(inputs)
    res = run_bass_kernel_spmd(nc, in_maps, list(range(N_CORES)))
    return _assemble(res.results)


# revision 14
# speedup vs baseline: 1.0137x; 1.0137x over previous
"""Trainium2 Bass kernel for nn_DivMergedLayer1 (dense_mlp, memory-bound).

The baked FFN weights are ultra-sparse: the whole module reduces to
``out = x`` everywhere except four scalars per batch row::

    op   = x[b, 0, 67]                      (opcode channel, >= 0)
    sg   = sum_i f32(f32(60*op) * f32(2^i * x[b, i, 0])) / 60
    s2   = sum_i max((x[b,i,1] > 0.5) * (2^i * x[b,i,1]), exp(-60))
    out[b, 0, k] = x[b,0,k] + f32(60*op * x[b,0,k]) * (-1/60)   k in {2,3,4,5}
    out[b, 0, 2] += sg
    out[b, 0, 5] += op / s2

So the kernel is a memory-bound copy with a tiny per-row fixup.  The
output tolerance (2e-2 scale-relative on an output whose absmax is ~4e9)
lets the bulk copy ride in bfloat16: the host downcasts x once, the
device streams the bf16 image back out (halving HBM traffic vs f32),
and the four genuinely-computed scalars per row travel in a small f32
side-channel (exact slices of x) so the fixup math is bit-comparable to
a full-f32 kernel.  The host merges the f32 fixup column block into the
upcast bf16 copy.  Pure data parallel over the batch axis; 1024 rows
per core; ~17 MB of HBM traffic per core vs 33.5 MB for the f32 kernel.
"""

import math

import numpy as np

N_CORES = 8
B, N, D = 8192, 32, 128
F = N * D                  # 4096 flattened features per row
R = B // N_CORES           # 1024 rows per core
P = 128                    # SBUF partitions
RPP = R // P               # 8 rows per partition
C = 2 * N + 1 + 4          # side-channel floats per row: a[32] d[32] op slots[4]

OP_COL = 67                # flat index of opcode channel (pos 0, feat 64+3)

_INV_S = float(np.float32(1.0 / 60.0))
_NEG_INV_S = float(np.float32(-1.0 / 60.0))
_EXP_NEG60 = float(np.float32(math.exp(-60.0)))

VARIANT = "d2dP"   # DRAM->DRAM bulk copy on sync/scalar HWDGE rings; small
                   # side-channel/fix DMAs on the gpsimd SWDGE ring so the
                   # fix store never tail-ends a ring still draining copies
TILES = 8          # copy DMA chunks (must divide RPP*F partition free dim)

_COMPILED = None


def _build(variant=VARIANT, tiles=TILES):
    import concourse.bacc as bacc
    import concourse.mybir as mybir
    from concourse.tile import TileContext

    f32 = mybir.dt.float32
    bf16 = mybir.dt.bfloat16
    mult = mybir.AluOpType.mult
    add = mybir.AluOpType.add
    is_gt = mybir.AluOpType.is_gt
    amax = mybir.AluOpType.max

    nc = bacc.Bacc(
        "TRN2", target_bir_lowering=False, debug=False, num_devices=N_CORES
    )
    xb_h = nc.dram_tensor("xb", [R, F], bf16, kind="ExternalInput")
    sc_h = nc.dram_tensor("sc", [P, RPP * C], f32, kind="ExternalInput")
    pw_h = nc.dram_tensor("pw", [P, N], f32, kind="ExternalInput")
    ob_h = nc.dram_tensor("ob", [R, F], bf16, kind="ExternalOutput")
    fx_h = nc.dram_tensor("fx", [P, RPP * 4], f32, kind="ExternalOutput")

    # partition p holds rows p*RPP .. p*RPP+RPP-1 (contiguous per partition)
    xv = xb_h.ap().rearrange("(p j) f -> p (j f)", p=P)
    ov = ob_h.ap().rearrange("(p j) f -> p (j f)", p=P)
    assert (RPP * F) % tiles == 0
    CH = RPP * F // tiles      # chunk free-dim elements per partition

    with TileContext(nc) as tc:
        with (
            tc.tile_pool(name="const", bufs=1) as cpool,
            tc.tile_pool(name="big", bufs=2) as bpool,
            tc.tile_pool(name="small", bufs=4) as spool,
        ):
            # const + side-channel loads: keep them off the tail of a busy
            # ring.  "d2dP" stages them (and the fix store) on the gpsimd
            # SWDGE ring; others put them at the head of the scalar ring.
            small_eng = nc.gpsimd if variant == "d2dP" else nc.scalar
            pw = cpool.tile([P, N], f32)
            small_eng.dma_start(out=pw[:], in_=pw_h.ap())
            sct = cpool.tile([P, RPP, C], f32)
            small_eng.dma_start(
                out=sct[:], in_=sc_h.ap().rearrange("p (j c) -> p j c", c=C)
            )
            res = cpool.tile([P, RPP, 4], f32)

            # ---- bulk copy: out_bf16 = x_bf16, no compute dependency ----
            if variant == "d2dG":
                # sync/scalar take 3/8 each as `tiles` chunks; gpsimd takes
                # the last 2/8 as one single SWDGE instruction
                cut = (tiles * 3) // 8
                for t in range(tiles):
                    if t < cut:
                        eng = nc.sync
                    elif t < 2 * cut:
                        eng = nc.scalar
                    else:
                        break
                    eng.dma_start(
                        out=ov[:, t * CH:(t + 1) * CH],
                        in_=xv[:, t * CH:(t + 1) * CH],
                    )
                lo = 2 * cut * CH
                nc.gpsimd.dma_start(out=ov[:, lo:], in_=xv[:, lo:])
            elif variant.startswith("d2d"):
                engs = [nc.sync, nc.scalar]
                if variant == "d2d3":
                    engs.append(nc.gpsimd)
                elif variant == "d2d1":
                    engs = [nc.sync]
                deferred = []
                for t in range(tiles):
                    if variant == "d2dM" and t % 2 == 1 and t > tiles // 2:
                        deferred.append(t)
                        continue
                    engs[t % len(engs)].dma_start(
                        out=ov[:, t * CH:(t + 1) * CH],
                        in_=xv[:, t * CH:(t + 1) * CH],
                    )
            else:
                for t in range(tiles):
                    X = bpool.tile([P, CH], bf16, tag="X")
                    nc.sync.dma_start(out=X[:], in_=xv[:, t * CH:(t + 1) * CH])
                    nc.scalar.dma_start(
                        out=ov[:, t * CH:(t + 1) * CH], in_=X[:]
                    )

            # ---- fixup: per-row scalars from the f32 side-channel ----
            V = nc.vector
            for j in range(RPP):
                Bj = sct[:, j]
                a_ap = Bj[:, 0:N]
                d_ap = Bj[:, N:2 * N]
                op_ap = Bj[:, 2 * N:2 * N + 1]
                slots = Bj[:, 2 * N + 1:2 * N + 5]

                op60 = spool.tile([P, 1], f32, tag="op60")
                g = spool.tile([P, N], f32, tag="g")
                val = spool.tile([P, N], f32, tag="val")
                msk = spool.tile([P, N], f32, tag="msk")
                extra = spool.tile([P, 4], f32, tag="extra")
                s2 = spool.tile([P, 1], f32, tag="s2")
                s2r = spool.tile([P, 1], f32, tag="s2r")
                c4 = spool.tile([P, 4], f32, tag="c4")

                V.tensor_scalar_mul(op60[:], op_ap, 60.0)
                # gather term -> extra[:,0]
                V.tensor_tensor(g[:], a_ap, pw[:], mult)
                V.tensor_scalar_mul(g[:], g[:], op60[:])
                V.tensor_scalar(
                    g[:], g[:], _INV_S, None, mult, add,
                    accum_out=extra[:, 0:1],
                )
                # softmax1-reciprocal term -> extra[:,3]
                V.tensor_tensor(val[:], d_ap, pw[:], mult)
                V.tensor_scalar(msk[:], d_ap, 0.5, None, is_gt)
                V.tensor_tensor(val[:], val[:], msk[:], mult)
                V.tensor_scalar(
                    val[:], val[:], _EXP_NEG60, None, amax, add,
                    accum_out=s2[:],
                )
                V.reciprocal(s2r[:], s2[:])
                V.tensor_tensor(extra[:, 3:4], s2r[:], op_ap, mult)
                V.memset(extra[:, 1:3], 0.0)
                # cleared slots, matching the reference's rounding order
                V.tensor_scalar_mul(c4[:], slots, op60[:])
                V.scalar_tensor_tensor(c4[:], c4[:], _NEG_INV_S, slots, mult, add)
                V.tensor_tensor(res[:, j], c4[:], extra[:], add)
            fix_eng = {"d2dP": nc.gpsimd, "d2dM": nc.scalar}.get(variant, nc.sync)
            fix_eng.dma_start(
                out=fx_h.ap(), in_=res[:].rearrange("p j c -> p (j c)")
            )
            if variant == "d2dM":
                for t in deferred:
                    nc.scalar.dma_start(
                        out=ov[:, t * CH:(t + 1) * CH],
                        in_=xv[:, t * CH:(t + 1) * CH],
                    )
    nc.compile()
    return nc


def _get_compiled():
    global _COMPILED
    if _COMPILED is None:
        _COMPILED = _build()
    return _COMPILED


def _prep_in_maps(inputs):
    import ml_dtypes

    x = np.ascontiguousarray(np.asarray(inputs["x"], dtype=np.float32))
    assert x.shape == (B, N, D), x.shape
    xr = x.reshape(B, F)
    xb = xr.astype(ml_dtypes.bfloat16)
    bpw = np.asarray(inputs["base_powers"]).astype(np.float32)
    pw = np.ascontiguousarray(np.broadcast_to(bpw[None, :], (P, N)))
    sc = np.concatenate(
        [x[:, :, 0], x[:, :, 1], xr[:, OP_COL:OP_COL + 1], xr[:, 2:6]], axis=1
    )                                            # [B, C] f32
    in_maps = []
    for i in range(N_CORES):
        s = slice(i * R, (i + 1) * R)
        in_maps.append({
            "xb": np.ascontiguousarray(xb[s]),
            "sc": np.ascontiguousarray(sc[s]).reshape(P, RPP * C),
            "pw": pw,
        })
    return in_maps


def _assemble(results):
    outs = []
    for i in range(N_CORES):
        of = np.asarray(results[i]["ob"]).astype(np.float32)   # [R, F]
        fx = np.asarray(results[i]["fx"]).reshape(R, 4)
        of[:, 2:6] = fx
        outs.append(of)
    return np.ascontiguousarray(
        np.concatenate(outs, axis=0).reshape(B, N, D).astype(np.float32)
    )


def kernel(**inputs):
    from concourse.bass_utils import run_bass_kernel_spmd

    nc = _get_compiled()
    in_maps = _prep_in_maps(inputs)
    res = run_bass_kernel_spmd(nc, in_maps, list(range(N_CORES)))
    return _assemble(res.results)
